# revision 1
# baseline (speedup 1.0000x reference)
"""AtomPosGNN Trainium2 kernel: 4-layer GraphConv (norm='both') over a dense
0/1 adjacency, SPMD across 8 NeuronCores.

Sharding: nodes split 1024/core. Core m holds the full-height column block
A[:, m*1024:(m+1)*1024] (== row block transposed; A symmetric) as exact 0/1
bf16, resident in SBUF, split into an off-rank part "a" (7 rank blocks in
rotated order (m+1)%8, ..., (m+7)%8) and the local diagonal block "al".
Features z are all-gathered in bf16 and used as the stationary matmul operand,
so the aggregation produces hT = z_full^T @ A_blk which feeds the weight
matmul directly (no transposes). Degree norm r = 1/sqrt(max(deg,1)) rides the
feature side: pre-scale before each gather (layer 0 scales the gathered raw
features on the fly) and the dst scale folds into the PSUM eviction multiply.

Overlap structure:
- a dummy collective fires first so the one-time CC entry barrier overlaps the
  adjacency load;
- the input-feature AllGather runs unscaled immediately (hidden under the
  adjacency load); r is shared via a tiny AllGather;
- each layer's aggregation = local diagonal-block part (fed from SBUF, no
  comm) + 56 gathered chunks; the local part of the NEXT layer executes while
  this layer's output AllGather is in flight;
- each output AllGather is split into two column halves so the second half
  flies under the next layer's first gathered phase;
- gathered z rank-blocks are fetched with per-core dynamic (register) DMA
  offsets so every core skips its own rank block without branching.
"""

import numpy as np
import ml_dtypes

N = 8192
NCORES = 8
L = N // NCORES          # 1024 local nodes per core
EMB = 125
POS = 3
IN = 128                 # EMB + POS
H = 512
HH = H // 2              # column half for the split AllGather
RJ = L // 128            # 8 row chunks per core
NJ = L // 512            # 2 free-dim chunks of 512 in aggregation
NOTH = NCORES - 1        # 7 gathered (off-rank) blocks

BF16 = ml_dtypes.bfloat16

_STATE = {}


def _build(use_bias):
    import concourse.bass as bass
    import concourse.mybir as mybir
    import concourse.tile as tile
    from concourse import bacc
    from concourse.bass import ds
    from concourse.masks import make_identity

    f32 = mybir.dt.float32
    bf16 = mybir.dt.bfloat16
    u32 = mybir.dt.uint32
    EXP = mybir.ActivationFunctionType.Exp
    LN = mybir.ActivationFunctionType.Ln

    nc = bacc.Bacc("TRN2", target_bir_lowering=False, debug=False,
                   num_devices=NCORES)

    a_dram = nc.declare_dram_parameter("a", [N - L, L], bf16, isOutput=False)
    al_dram = nc.declare_dram_parameter("al", [L, L], bf16, isOutput=False)
    f0_dram = nc.declare_dram_parameter("f0", [L, IN], f32, isOutput=False)
    w0_dram = nc.declare_dram_parameter("w0", [IN, H], bf16, isOutput=False)
    wx_dram = [nc.declare_dram_parameter(f"w{i}", [H, H], bf16, isOutput=False)
               for i in (1, 2, 3)]
    b_dram = nc.declare_dram_parameter("b", [4, H], bf16, isOutput=False)
    ko_dram = nc.declare_dram_parameter("ko", [1, 8], u32, isOutput=False)
    out_dram = nc.declare_dram_parameter("out", [L, H], f32, isOutput=True)

    rg = [list(range(NCORES))]

    def allgather(ins_ap, outs_ap):
        nc.gpsimd.collective_compute(
            "AllGather", mybir.AluOpType.bypass, replica_groups=rg,
            ins=[ins_ap], outs=[outs_ap])

    with tile.TileContext(nc) as tc:
        with (
            tc.tile_pool(name="sb", bufs=1) as sb,
            tc.tile_pool(name="zp", bufs=2) as zp,
            tc.tile_pool(name="hp", bufs=4) as hp,
            tc.tile_pool(name="ep", bufs=2) as ep,
            tc.tile_pool(name="zc", bufs=16) as zcp,
            tc.tile_pool(name="ps", bufs=8, space="PSUM") as ps,
            tc.tile_pool(name="dr", bufs=1, space="DRAM") as dr,
        ):
            # ---- immediate: AllGather raw bf16 input features (this is the
            # first collective, so the one-time CC entry barrier overlaps the
            # adjacency load) ----
            ag_f0i = dr.tile([L, IN], bf16, tag="agf0i")
            ag_f0o = dr.tile([N, IN], bf16, tag="agf0o", addr_space="Shared")
            f0cs = []
            for rj in range(RJ):
                f0c = zcp.tile([128, IN], f32, tag="f0c", bufs=RJ, name=f"f0c{rj}")
                nc.sync.dma_start(f0c[:], f0_dram[rj * 128:(rj + 1) * 128, :])
                f0cs.append(f0c)

            # ---- persistent SBUF tiles / adjacency load ----
            a_sb = sb.tile([128, NOTH * RJ, L], bf16)     # 112 KB/partition
            al_sb = sb.tile([128, RJ, L], bf16)           # 16 KB/partition
            for k in range(NOTH * RJ):
                nc.sync.dma_start(a_sb[:, k, :], a_dram[k * 128:(k + 1) * 128, :])
            for k in range(RJ):
                nc.sync.dma_start(al_sb[:, k, :], al_dram[k * 128:(k + 1) * 128, :])

            ones_col = sb.tile([128, 1], bf16)            # deg lhsT
            ones_row_b = sb.tile([1, 128], bf16)          # bias lhsT
            r_bcast = sb.tile([128, L], f32)              # dst scale, local rows
            r_pp = sb.tile([128, RJ], f32)                # local r per-partition
            ident64 = sb.tile([64, 64], f32)
            nc.vector.memset(ones_col[:], 1.0)
            nc.vector.memset(ones_row_b[:], 1.0)
            make_identity(nc, ident64[:])

            # per-core gathered-block row offsets (rotated rank order; [7]=own)
            koff = []
            for j in range(NCORES):
                rko = nc.sync.alloc_register(f"rko{j}")
                nc.sync.reg_load(rko, ko_dram[0:1, j:j + 1])
                koff.append(nc.sync.snap(rko, donate=True, min_val=0,
                                         max_val=N - L))

            # ---- degree of local nodes: colsums of the local column block ----
            deg_ps = [ps.tile([1, 512], f32, tag="acc", name=f"degps{j}")
                      for j in range(NJ)]
            for k in range(NOTH * RJ + RJ):
                src = a_sb[:, k, :] if k < NOTH * RJ else al_sb[:, k - NOTH * RJ, :]
                for j in range(NJ):
                    nc.tensor.matmul(deg_ps[j][:], ones_col[:],
                                     src[:, j * 512:(j + 1) * 512],
                                     start=(k == 0), stop=(k == NOTH * RJ + RJ - 1))
            t0 = sb.tile([1, L], f32)
            r_row = sb.tile([1, L], f32)
            for j in range(NJ):
                nc.scalar.copy(t0[:, j * 512:(j + 1) * 512], deg_ps[j][:])
            # r = sqrt(1/max(deg,1))
            nc.vector.tensor_scalar_max(r_row[:], t0[:], 1.0)
            nc.vector.reciprocal(t0[:], r_row[:])
            nc.scalar.sqrt(r_row[:], t0[:])

            # local r: broadcast across partitions (dst scale) + per-partition
            ones_row_f = sb.tile([1, 128], f32)
            nc.vector.memset(ones_row_f[:], 1.0)
            for j in range(NJ):
                rb_ps = ps.tile([128, 512], f32, tag="acc", name=f"rbps{j}")
                nc.tensor.matmul(rb_ps[:], ones_row_f[:],
                                 r_row[:, j * 512:(j + 1) * 512],
                                 start=True, stop=True)
                nc.vector.tensor_copy(r_bcast[:, j * 512:(j + 1) * 512], rb_ps[:])
            # r per-partition for own rows: natural [8, 128] reload of r, then
            # one PE transpose (an element-strided [128, 8] DMA costs ~10-15us)
            r_dram = dr.tile([1, L], f32, tag="rd")
            nc.sync.dma_start(r_dram[:], r_row[:])
            r_nat8 = sb.tile([8, 128], f32)
            nc.sync.dma_start(r_nat8[:],
                              r_dram[0].rearrange("(a b) -> a b", a=RJ))
            rp_ps = ps.tile([128, RJ], f32, tag="acc", name="rpps")
            nc.tensor.transpose(rp_ps[:], r_nat8[:], ident64[0:RJ, 0:RJ])
            nc.vector.tensor_copy(r_pp[:], rp_ps[:])

            # layer-0 stationary operand z0 = r * f0 (bf16): local lhsT tiles
            # AND the AllGather payload (first collective — the CC entry
            # barrier runs under the adjacency load and gates it to ~73us
            # regardless, which is when r is ready)
            zl0 = []
            for rj in range(RJ):
                z = zcp.tile([128, IN], bf16, tag="zl0", bufs=RJ, name=f"zl0{rj}")
                nc.vector.tensor_scalar_mul(z[:], f0cs[rj][:], r_pp[:, rj:rj + 1])
                nc.sync.dma_start(ag_f0i[rj * 128:(rj + 1) * 128, :], z[:])
                zl0.append(z)
            allgather(ag_f0i[:], ag_f0o[:])

            # ---- weights (needed only ~100us in) ----
            w0_sb = sb.tile([128, 1, H], bf16)
            wx_sb = [sb.tile([128, 4, H], bf16, name=f"wx{i}") for i in range(3)]
            b_sb = sb.tile([1, 4, H], bf16)
            nc.sync.dma_start(w0_sb[:, 0, :], w0_dram[:])
            for i in range(3):
                for ci in range(4):
                    nc.sync.dma_start(wx_sb[i][:, ci, :],
                                      wx_dram[i][ci * 128:(ci + 1) * 128, :])
            for l in range(4):
                nc.sync.dma_start(b_sb[:, l, :], b_dram[l:l + 1, :])

            # ---- layers ----
            # local_z: per rj the stationary tiles of this core's own rows
            # (layer 0: one [128, IN] tile; later: two [128, HH] half tiles).
            # zsrc: gathered buffers (layer 0: one full-width; later: halves).
            local_z = [(z,) for z in zl0]
            zsrc = [ag_f0o]
            cin = IN
            for layer in range(4):
                ci_n = cin // 128
                n_ph = len(zsrc)
                ci_per = ci_n // n_ph
                zw = cin // n_ph               # gathered buffer width

                # all psum accumulators for this layer's aggregation
                h_ps = [[ps.tile([128, 512], f32, tag="acc",
                                 name=f"hps{layer}_{ci}_{j}")
                         for j in range(NJ)] for ci in range(ci_n)]

                # local part: this core's diagonal block, no comm needed —
                # fills the window while the output AllGathers of the previous
                # layer are in flight
                for rj in range(RJ):
                    for ci in range(ci_n):
                        zt = local_z[rj][ci // ci_per]
                        for j in range(NJ):
                            nc.tensor.matmul(
                                h_ps[ci][j][:],
                                zt[:, (ci % ci_per) * 128:
                                   (ci % ci_per + 1) * 128],
                                al_sb[:, rj, j * 512:(j + 1) * 512],
                                start=(rj == 0), stop=False)

                # gathered part: 7 off-rank blocks per phase, fetched with
                # per-core dynamic offsets (own block skipped by construction)
                hT = [hp.tile([128, L], bf16, tag="hT", name=f"hT{layer}_{x}")
                      for x in range(ci_n)]
                for ph in range(n_ph):
                    zbuf = zsrc[ph]
                    for j in range(NOTH):
                        zkb = zp.tile([128, RJ, zw], bf16, tag="zkb")
                        nc.sync.dma_start(
                            zkb[:],
                            zbuf[ds(koff[j], L), :].rearrange(
                                "(c p) w -> p c w", p=128))
                        for c in range(RJ):
                            s = j * RJ + c
                            zt = zkb[:, c, :]
                            last = (j == NOTH - 1) and (c == RJ - 1)
                            for cl in range(ci_per):
                                ci = ph * ci_per + cl
                                for nj in range(NJ):
                                    nc.tensor.matmul(
                                        h_ps[ci][nj][:],
                                        zt[:, cl * 128:(cl + 1) * 128],
                                        a_sb[:, s, nj * 512:(nj + 1) * 512],
                                        start=False, stop=last)
                    # evict this phase's channels (dst scale folded in)
                    for nj in range(NJ):
                        for cl in range(ci_per):
                            ci = ph * ci_per + cl
                            nc.vector.tensor_mul(
                                hT[ci][:, nj * 512:(nj + 1) * 512],
                                h_ps[ci][nj][:],
                                r_bcast[:, nj * 512:(nj + 1) * 512])

                # weight matmul + bias + softplus in two column halves; the
                # first half's AllGather overlaps the second half's compute and
                # the next layer's local aggregation
                if layer < 3:
                    ag_i = [dr.tile([L, HH], bf16, tag=f"agi{layer}_{hf}",
                                    name=f"agi{layer}_{hf}")
                            for hf in range(2)]
                    ag_o = [dr.tile([N, HH], bf16, tag=f"ago{layer}_{hf}",
                                    addr_space="Shared",
                                    name=f"ago{layer}_{hf}") for hf in range(2)]
                w_l = w0_sb if layer == 0 else wx_sb[layer - 1]
                new_local = [[None, None] for _ in range(RJ)]
                # layer 3 has no output AllGather, so no need for the column
                # halves — full-width epilogue halves the ACT op count
                n_oph = 2 if layer < 3 else 1
                wo = H // n_oph
                for hf in range(n_oph):
                    cs = slice(hf * wo, (hf + 1) * wo)
                    for rj in range(RJ):
                        y_ps = ps.tile([128, wo], f32, tag="acc",
                                       name=f"yps{layer}_{hf}_{rj}")
                        if use_bias:
                            nc.tensor.matmul(y_ps[:], ones_row_b[:],
                                             b_sb[:, layer, cs],
                                             start=True, stop=False)
                        for ci in range(ci_n):
                            nc.tensor.matmul(y_ps[:],
                                             hT[ci][:, rj * 128:(rj + 1) * 128],
                                             w_l[:, ci, cs],
                                             start=(ci == 0 and not use_bias),
                                             stop=(ci == ci_n - 1))
                        # softplus = ln(exp(y) + 1); table lacks native Softplus
                        ey = ep.tile([128, wo], f32, tag="ey")
                        nc.scalar.activation(ey[:], y_ps[:], EXP)
                        sp = ep.tile([128, wo], f32, tag="sp")
                        nc.scalar.activation(sp[:], ey[:], LN, bias=1.0)
                        if layer < 3:
                            zc = zcp.tile([128, HH], bf16, tag="zo",
                                          name=f"zc{layer}_{hf}_{rj}")
                            nc.vector.tensor_scalar_mul(zc[:], sp[:],
                                                        r_pp[:, rj:rj + 1])
                            nc.sync.dma_start(
                                ag_i[hf][rj * 128:(rj + 1) * 128, :], zc[:])
                            new_local[rj][hf] = zc
                        else:
                            nc.sync.dma_start(
                                out_dram[rj * 128:(rj + 1) * 128, cs], sp[:])
                    if layer < 3:
                        allgather(ag_i[hf][:], ag_o[hf][:])
                if layer < 3:
                    local_z = [tuple(t) for t in new_local]
                    zsrc = ag_o
                    cin = H

    nc.compile()
    return nc


def _prep_shards(atom_pos, dist_adj, atom_emb, W0, b0, W1, b1, W2, b2, W3, b3):
    adj = np.asarray(dist_adj, dtype=np.float32).copy()
    np.fill_diagonal(adj, 0.0)          # reference removes self loops
    a_bf = adj.astype(BF16)             # entries are exactly 0/1
    feat0 = np.concatenate(
        [np.asarray(atom_emb, np.float32), np.asarray(atom_pos, np.float32)],
        axis=1)
    w0 = np.asarray(W0, np.float32).astype(BF16)
    wx = [np.asarray(w, np.float32).astype(BF16) for w in (W1, W2, W3)]
    b = np.stack([np.asarray(x, np.float32) for x in (b0, b1, b2, b3)]
                 ).astype(BF16)
    in_maps = []
    for m in range(NCORES):
        sl = slice(m * L, (m + 1) * L)
        blk = a_bf[:, sl]
        rot = [(m + 1 + j) % NCORES for j in range(NOTH)]
        a_oth = np.concatenate([blk[r * L:(r + 1) * L] for r in rot], axis=0)
        ko = np.array([[r * L for r in rot] + [m * L]], dtype=np.uint32)
        im = {"a": np.ascontiguousarray(a_oth),
              "al": np.ascontiguousarray(blk[m * L:(m + 1) * L]),
              "f0": np.ascontiguousarray(feat0[sl]),
              "w0": w0, "w1": wx[0], "w2": wx[1], "w3": wx[2], "b": b,
              "ko": ko}
        in_maps.append(im)
    return in_maps


def kernel(**inputs):
    from concourse.bass_utils import run_bass_kernel_spmd

    use_bias = any(
        np.any(np.asarray(inputs[f"b{i}"]) != 0) for i in range(4))
    key = ("nc", use_bias)
    if key not in _STATE:
        _STATE[key] = _build(use_bias)
    nc = _STATE[key]
    in_maps = _prep_shards(**inputs)
    res = run_bass_kernel_spmd(nc, in_maps, core_ids=list(range(NCORES)))
    out = np.concatenate([res.results[m]["out"] for m in range(NCORES)], axis=0)
    return out.astype(np.float32)



# revision 3
# speedup vs baseline: 1.7543x; 1.7543x over previous
"""AtomPosGNN Trainium2 kernel: 4-layer GraphConv (norm='both') over a dense
0/1 adjacency, SPMD across 8 NeuronCores.

Sharding: nodes split 1024/core. Core m holds the full-height column block
A[:, m*1024:(m+1)*1024] (== row block transposed; A symmetric) as exact 0/1
fp8e4m3, resident in SBUF, rows reordered into rotated rank order
(m+1)%8, ..., (m+7)%8, m (own block last). The aggregation runs as fp8
DoubleRow matmuls (two 128-row source chunks per instruction, 2x PE
throughput): stationary operand = feature chunk pair [128, 2, 128ch],
moving operand = adjacency chunk pair [128, 2, 512dst].

Host precomputes the degree norms r = rsqrt(max(deg,1)) (graph setup, same
as dgl) and the pre-scaled input features z0 = 16*r*[emb|pos] in fp8,
replicated to every core in its rotated row order — so layer 0 needs no
collective at all and starts immediately. Features are scaled by 16 to keep
fp8 values out of the subnormal range; the 1/16 is folded into the dst-scale
vector rbc. A tiny dummy AllGather fires first so the one-time CC entry
barrier overlaps the adjacency load and layer 0.

Layers 1-3 gather features via fp8 AllGathers split into two column halves;
the second half flies under the first half's aggregation phase, and the next
layer's local (own-block) aggregation runs while the collectives are in
flight. Gathered rank blocks are fetched with per-core dynamic (register)
DMA offsets so every core skips its own block without branching.

softplus = ln(exp(y)+1) on the ACT engine; the activation-table selection is
patched so both EXP and LN resolve to the combined natural_log_exp table —
one table load for the whole kernel instead of a ~1.3us reload per function
switch.
"""

import numpy as np
import ml_dtypes

N = 8192
NCORES = 8
L = N // NCORES          # 1024 local nodes per core
EMB = 125
POS = 3
IN = 128                 # EMB + POS
H = 512
HH = H // 2              # column half for the split AllGather
RJ = L // 128            # 8 row chunks per rank block
NJ = L // 512            # 2 free-dim chunks of 512 in aggregation
NOTH = NCORES - 1        # 7 gathered (off-rank) blocks
NCH = N // 128           # 64 source chunks total
ZS = 16.0                # fp8 feature scale (keeps z out of subnormals)

BF16 = ml_dtypes.bfloat16
F8 = ml_dtypes.float8_e4m3fn

_STATE = {}


def _patch_act_tables():
    """Make the act-table pass pick the combined exp+ln table for both EXP
    and LN (greedy first-match otherwise alternates two tables, reloading
    ~1.3us per switch). Empties the tables before the combined one so ids
    stay positional."""
    import concourse.bacc as bm
    import concourse.hw_specs as hw
    if getattr(bm, "_gnn_act_patch", False):
        return
    orig = hw.get_activation_tables

    def patched(arch):
        t = orig(arch)
        names = list(t.keys())
        if "natural_log_exp_and_others" not in names:
            return t
        i = names.index("natural_log_exp_and_others")
        return {n: (set() if k < i else t[n]) for k, n in enumerate(names)}

    bm.get_activation_tables = patched
    bm._gnn_act_patch = True


def _build(use_bias):
    import concourse.bass as bass
    import concourse.mybir as mybir
    import concourse.tile as tile
    from concourse import bacc
    from concourse.bass import ds

    _patch_act_tables()

    f32 = mybir.dt.float32
    bf16 = mybir.dt.bfloat16
    fp8 = mybir.dt.float8e4
    u32 = mybir.dt.uint32
    EXP = mybir.ActivationFunctionType.Exp
    LN = mybir.ActivationFunctionType.Ln
    DR = mybir.MatmulPerfMode.DoubleRow

    nc = bacc.Bacc("TRN2", target_bir_lowering=False, debug=False,
                   num_devices=NCORES)

    a_dram = nc.declare_dram_parameter("a", [N, L], fp8, isOutput=False)
    z0_dram = nc.declare_dram_parameter("z0", [N, IN], fp8, isOutput=False)
    rbc_dram = nc.declare_dram_parameter("rbc", [128, L], f32, isOutput=False)
    rp_dram = nc.declare_dram_parameter("rp", [128, RJ], f32, isOutput=False)
    w0_dram = nc.declare_dram_parameter("w0", [IN, H], bf16, isOutput=False)
    wx_dram = [nc.declare_dram_parameter(f"w{i}", [H, H], bf16, isOutput=False)
               for i in (1, 2, 3)]
    b_dram = nc.declare_dram_parameter("b", [4, H], bf16, isOutput=False)
    ko_dram = nc.declare_dram_parameter("ko", [1, 8], u32, isOutput=False)
    out_dram = nc.declare_dram_parameter("out", [L, H], f32, isOutput=True)

    rg = [list(range(NCORES))]

    def allgather(ins_ap, outs_ap):
        nc.gpsimd.collective_compute(
            "AllGather", mybir.AluOpType.bypass, replica_groups=rg,
            ins=[ins_ap], outs=[outs_ap])

    with tile.TileContext(nc) as tc:
        with (
            tc.tile_pool(name="sb", bufs=1) as sb,
            tc.tile_pool(name="zp", bufs=2) as zp,
            tc.tile_pool(name="lz", bufs=2) as lzp,
            tc.tile_pool(name="hp", bufs=4) as hp,
            tc.tile_pool(name="ep", bufs=4) as ep,
            tc.tile_pool(name="ps", bufs=8, space="PSUM") as ps,
            tc.tile_pool(name="dr", bufs=1, space="DRAM") as dr,
        ):
            # ---- dummy first collective: absorbs the one-time CC entry
            # barrier under the adjacency load + layer 0 ----
            ag_di = dr.tile([1, 8], u32, tag="agdi")
            ag_d = dr.tile([NCORES, 8], u32, tag="agd", addr_space="Shared")
            nc.sync.dma_start(ag_di[:], ko_dram[:])
            allgather(ag_di[:], ag_d[:])

            # ---- persistent SBUF tiles / loads ----
            # z0 + adjacency interleaved so layer 0 can start on chunk pair 0
            # while the rest still streams in
            a_sb = sb.tile([128, NCH, L], fp8)        # 64 KB/partition
            z0_sb = sb.tile([128, NCH, IN], fp8)      # 8 KB/partition
            for g in range(NCORES):
                nc.sync.dma_start(
                    z0_sb[:, g * RJ:(g + 1) * RJ, :],
                    z0_dram[g * L:(g + 1) * L, :].rearrange(
                        "(c p) w -> p c w", p=128))
                for c in range(RJ):
                    k = g * RJ + c
                    nc.sync.dma_start(a_sb[:, k, :],
                                      a_dram[k * 128:(k + 1) * 128, :])

            rbc = sb.tile([128, L], f32)              # dst scale (has 1/ZS)
            rp = sb.tile([128, RJ], f32)              # src scale (has ZS)
            nc.sync.dma_start(rbc[:], rbc_dram[:])
            nc.sync.dma_start(rp[:], rp_dram[:])

            # per-core gathered-block row offsets (rotated rank order)
            koff = []
            for j in range(NOTH):
                rko = nc.sync.alloc_register(f"rko{j}")
                nc.sync.reg_load(rko, ko_dram[0:1, j:j + 1])
                koff.append(nc.sync.snap(rko, donate=True, min_val=0,
                                         max_val=N - L))

            # ---- weights (needed only once layer 0's aggregation is done) ----
            w0_sb = sb.tile([128, 1, H], bf16)
            wx_sb = [sb.tile([128, 4, H], bf16, name=f"wx{i}") for i in range(3)]
            nc.sync.dma_start(w0_sb[:, 0, :], w0_dram[:])
            for i in range(3):
                for ci in range(4):
                    nc.sync.dma_start(wx_sb[i][:, ci, :],
                                      wx_dram[i][ci * 128:(ci + 1) * 128, :])
            if use_bias:
                b_sb = sb.tile([1, 4, H], bf16)
                ones_row_b = sb.tile([1, 128], bf16)
                nc.vector.memset(ones_row_b[:], 1.0)
                for l in range(4):
                    nc.sync.dma_start(b_sb[:, l, :], b_dram[l:l + 1, :])

            # ---- layer 0 aggregation: all 64 chunks local (z0 replicated) ----
            h0 = [ps.tile([128, 512], f32, tag="acc", name=f"h0_{j}")
                  for j in range(NJ)]
            for t in range(NCH // 2):
                for j in range(NJ):
                    nc.tensor.matmul(
                        h0[j][:],
                        z0_sb[:, 2 * t:2 * t + 2, :],
                        a_sb[:, 2 * t:2 * t + 2, j * 512:(j + 1) * 512],
                        start=(t == 0), stop=(t == NCH // 2 - 1),
                        perf_mode=DR)
            hT0 = hp.tile([128, 1, L], bf16, tag="hT", name="hT0")
            for j in range(NJ):
                nc.vector.tensor_mul(hT0[:, 0, j * 512:(j + 1) * 512],
                                     h0[j][:], rbc[:, j * 512:(j + 1) * 512])

            # ---- layers ----
            lz = None          # local z tile [128, RJ, H] fp8 (layers 1+)
            hT = [hT0]         # aggregation output chunks, bf16
            zsrc = None        # gathered halves (DRAM, Shared)
            for layer in range(4):
                if layer > 0:
                    ci_n = 4
                    # local part: own-block aggregation, no comm needed —
                    # runs while the output AllGathers are in flight
                    h_ps = [[ps.tile([128, 512], f32, tag="acc",
                                     name=f"hps{layer}_{ci}_{j}")
                             for j in range(NJ)] for ci in range(ci_n)]
                    for t in range(RJ // 2):
                        for ci in range(ci_n):
                            for j in range(NJ):
                                nc.tensor.matmul(
                                    h_ps[ci][j][:],
                                    lz[:, 2 * t:2 * t + 2,
                                       ci * 128:(ci + 1) * 128],
                                    a_sb[:, NOTH * RJ + 2 * t:
                                         NOTH * RJ + 2 * t + 2,
                                         j * 512:(j + 1) * 512],
                                    start=(t == 0), stop=False,
                                    perf_mode=DR)
                    # gathered part: 7 off-rank blocks per half-phase
                    hT = [hp.tile([128, 1, L], bf16, tag="hT",
                                  name=f"hT{layer}_{x}") for x in range(ci_n)]
                    for ph in range(2):
                        for j in range(NOTH):
                            zkb = zp.tile([128, RJ, HH], fp8, tag="zkb")
                            nc.sync.dma_start(
                                zkb[:],
                                zsrc[ph][ds(koff[j], L), :].rearrange(
                                    "(c p) w -> p c w", p=128))
                            for t in range(RJ // 2):
                                last = (j == NOTH - 1) and (t == RJ // 2 - 1)
                                for cl in range(2):
                                    ci = ph * 2 + cl
                                    for nj in range(NJ):
                                        nc.tensor.matmul(
                                            h_ps[ci][nj][:],
                                            zkb[:, 2 * t:2 * t + 2,
                                                cl * 128:(cl + 1) * 128],
                                            a_sb[:, j * RJ + 2 * t:
                                                 j * RJ + 2 * t + 2,
                                                 nj * 512:(nj + 1) * 512],
                                            start=False, stop=last,
                                            perf_mode=DR)
                        # evict this phase's channels (dst scale folded in)
                        for cl in range(2):
                            ci = ph * 2 + cl
                            for nj in range(NJ):
                                nc.vector.tensor_mul(
                                    hT[ci][:, 0, nj * 512:(nj + 1) * 512],
                                    h_ps[ci][nj][:],
                                    rbc[:, nj * 512:(nj + 1) * 512])
                else:
                    ci_n = 1

                # weight matmul + softplus epilogue; layers 0-2 in two column
                # halves so the first half's AllGather overlaps the second
                # half and the next layer's local aggregation
                if layer < 3:
                    ag_i = [dr.tile([L, HH], fp8, tag=f"agi{layer}_{hf}",
                                    name=f"agi{layer}_{hf}")
                            for hf in range(2)]
                    ag_o = [dr.tile([N, HH], fp8, tag=f"ago{layer}_{hf}",
                                    addr_space="Shared",
                                    name=f"ago{layer}_{hf}") for hf in range(2)]
                    lzn = lzp.tile([128, RJ, H], fp8, tag="lz",
                                   name=f"lz{layer}")
                w_l = w0_sb if layer == 0 else wx_sb[layer - 1]
                n_oph = 2 if layer < 3 else 1
                wo = H // n_oph
                for hf in range(n_oph):
                    cs = slice(hf * wo, (hf + 1) * wo)
                    for rj in range(RJ):
                        y_ps = ps.tile([128, wo], f32, tag="acc",
                                       name=f"yps{layer}_{hf}_{rj}")
                        if use_bias:
                            nc.tensor.matmul(y_ps[:], ones_row_b[:],
                                             b_sb[:, layer, cs],
                                             start=True, stop=False)
                        for ci in range(ci_n):
                            nc.tensor.matmul(y_ps[:],
                                             hT[ci][:, 0,
                                                    rj * 128:(rj + 1) * 128],
                                             w_l[:, ci, cs],
                                             start=(ci == 0 and not use_bias),
                                             stop=(ci == ci_n - 1))
                        # softplus = ln(exp(y) + 1)
                        ey = ep.tile([128, wo], f32, tag="ey")
                        nc.scalar.activation(ey[:], y_ps[:], EXP)
                        if layer < 3:
                            sp = ep.tile([128, wo], f32, tag="sp")
                            nc.scalar.activation(sp[:], ey[:], LN, bias=1.0)
                            nc.vector.tensor_scalar_mul(
                                lzn[:, rj, cs], sp[:], rp[:, rj:rj + 1])
                            nc.sync.dma_start(
                                ag_i[hf][rj * 128:(rj + 1) * 128, :],
                                lzn[:, rj, cs])
                        else:
                            sp = ep.tile([128, wo], f32, tag="sp")
                            nc.scalar.activation(sp[:], ey[:], LN, bias=1.0)
                            nc.sync.dma_start(
                                out_dram[rj * 128:(rj + 1) * 128, :], sp[:])
                    if layer < 3:
                        allgather(ag_i[hf][:], ag_o[hf][:])
                if layer < 3:
                    lz = lzn
                    zsrc = ag_o

    nc.compile()
    return nc


def _prep_shards(atom_pos, dist_adj, atom_emb, W0, b0, W1, b1, W2, b2, W3, b3):
    adj = np.asarray(dist_adj, dtype=np.float32).copy()
    np.fill_diagonal(adj, 0.0)          # reference removes self loops
    deg = adj.sum(axis=0)               # symmetric: in-deg == out-deg
    r = 1.0 / np.sqrt(np.maximum(deg, 1.0))
    feat0 = np.concatenate(
        [np.asarray(atom_emb, np.float32), np.asarray(atom_pos, np.float32)],
        axis=1)
    z0 = (ZS * r[:, None] * feat0).astype(F8)     # pre-scaled input features
    a8 = adj.astype(F8)                           # entries exactly 0/1
    w0 = np.asarray(W0, np.float32).astype(BF16)
    wx = [np.asarray(w, np.float32).astype(BF16) for w in (W1, W2, W3)]
    b = np.stack([np.asarray(x, np.float32) for x in (b0, b1, b2, b3)]
                 ).astype(BF16)
    in_maps = []
    for m in range(NCORES):
        sl = slice(m * L, (m + 1) * L)
        rot = [(m + 1 + j) % NCORES for j in range(NOTH)] + [m]
        rows = np.concatenate([np.arange(rk * L, (rk + 1) * L) for rk in rot])
        r_loc = r[sl].astype(np.float32)
        rbc = np.broadcast_to(r_loc / ZS, (128, L)).copy()   # dst scale
        rp = (ZS * r_loc).reshape(RJ, 128).T.copy()          # src scale
        ko = np.array([[rk * L for rk in rot]], dtype=np.uint32)
        im = {"a": np.ascontiguousarray(a8[rows][:, sl]),
              "z0": np.ascontiguousarray(z0[rows]),
              "rbc": rbc, "rp": rp,
              "w0": w0, "w1": wx[0], "w2": wx[1], "w3": wx[2], "b": b,
              "ko": ko}
        in_maps.append(im)
    return in_maps


def kernel(**inputs):
    from concourse.bass_utils import run_bass_kernel_spmd

    use_bias = any(
        np.any(np.asarray(inputs[f"b{i}"]) != 0) for i in range(4))
    key = ("nc", use_bias)
    if key not in _STATE:
        _STATE[key] = _build(use_bias)
    nc = _STATE[key]
    in_maps = _prep_shards(**inputs)
    res = run_bass_kernel_spmd(nc, in_maps, core_ids=list(range(NCORES)))
    out = np.concatenate([res.results[m]["out"] for m in range(NCORES)], axis=0)
    return out.astype(np.float32)
